# revision 57
# baseline (speedup 1.0000x reference)
"""Distributed multi-head attention kernel for 8 TRN2 NeuronCores.

Problem: B=2, N=2048, C=768, H=12 heads of dim 64.
  q = x @ Wq.T ; k = x @ Wk.T ; v = x @ Wv.T      (per-head split)
  out = softmax(q k^T / 8) v                        (full N^2 attention)
  y = concat_heads(out) @ Wo.T + bo

Sharding: 24 (batch, head) pairs -> 3 per core.  Core i owns batch i//4 and
heads 3*(i%4)..3*(i%4)+2.  Projections + attention are fully local (weights
row-sliced on the host).  An 8-way AllToAll then redistributes the per-head
context so core i owns query rows 256*i..256*(i+1) of BOTH batches with all
12 heads, after which the output projection (full Wo, replicated) produces a
disjoint output slice per core.  All matmuls run in bf16 (f32 PSUM accum).
"""

import numpy as np
import ml_dtypes

import concourse.bass as bass
import concourse.mybir as mybir
import concourse.tile as tile
from concourse import bacc
from concourse.bass_utils import run_bass_kernel_spmd

B, N, C, H, HD = 2, 2048, 768, 12, 64
SCALE = HD ** -0.5          # 0.125
P = 128
CB = C // P                 # 6 contraction blocks of 128 over channels
KB = N // P                 # 16 key blocks
QCH = 512                   # query chunk (max moving free dim)
NQC = N // QCH              # 4
HPC = 3                     # heads per core
NCORES = 8
VW = HPC * (HD + 1)         # 195: v columns per key-block (3 heads + ones col)
RQ = N // NCORES            # 256 query rows per core per batch after A2A

f32 = mybir.dt.float32
bf16 = mybir.dt.bfloat16
Exp = mybir.ActivationFunctionType.Exp
Identity = mybir.ActivationFunctionType.Identity

# head -> (block, partition offset) inside qT_sb / kT_sb [128, 2*2048].
# Identical offsets for q and k per head (PE needs matching base partitions):
# block 0 rows 0:64 = head 0, rows 64:128 = head 1; block 1 rows 0:64 = head 2.
HOFF = {0: (0, 0), 1: (0, 64), 2: (1, 0)}
# wqkT host column order: [q0 q1 | k0 k1 | q2 | k2] (projection passes)
PROJ_PASSES = [
    # (wqkT col offset, M, dest 'q' or 'k', dest block)
    (0, 128, "q", 0),
    (128, 128, "k", 0),
    (256, 64, "q", 1),
    (320, 64, "k", 1),
]


def _body(nc, tc, xT, wqkT, wvT, woT, bo_d, out_d, dbg=None):
    with (
        tc.tile_pool(name="const", bufs=1) as constp,
        tc.tile_pool(name="big", bufs=1) as bigp,
        tc.tile_pool(name="esp", bufs=8) as esp,
        tc.tile_pool(name="smallp", bufs=4) as smallp,
        tc.tile_pool(name="outp", bufs=2) as outp,
        tc.tile_pool(name="psS", bufs=2, space="PSUM") as psS,
        tc.tile_pool(name="psC", bufs=4, space="PSUM") as psC,
        tc.tile_pool(name="dram", bufs=1, space="DRAM") as dramp,
    ):
        # psS tiles are [128, 1024] f32 (2 banks); also reused for the
        # projection, broadcast, and output-projection matmuls.
        # psC: 4 single-bank PV accumulators.  Total 2*2 + 4 = 8 banks.
        # ---- load inputs to SBUF (all bf16 except bias) ----
        xT_sb = bigp.tile([P, CB * N], bf16, name="xT_sb")
        wqkT_sb = bigp.tile([P, CB * 384], bf16, name="wqkT_sb")
        wvT_sb = bigp.tile([P, CB * 192], bf16, name="wvT_sb")
        woT_sb = bigp.tile([P, CB * C], bf16, name="woT_sb")
        bo_sb = bigp.tile([P, CB], f32, name="bo_sb")
        ones_sb = constp.tile([P, 64], bf16, name="ones_sb")
        nc.vector.memset(ones_sb[:, :], 1.0)
        # warm the ACT exp table (one-time ~2.7us PSEUDO_LOAD) during loads
        warm_sb = constp.tile([P, 1], f32, name="warm_sb")
        nc.scalar.activation(warm_sb[0:1, :], ones_sb[0:1, 0:1], Exp, scale=SCALE)
        # weights for the first projection pass and x first; wo/bo last
        for cb in range(CB):
            nc.sync.dma_start(wqkT_sb[:, cb * 384:(cb + 1) * 384], wqkT[cb * P:(cb + 1) * P, :])
        for cb in range(CB):
            nc.sync.dma_start(xT_sb[:, cb * N:(cb + 1) * N], xT[cb * P:(cb + 1) * P, :])
            nc.sync.dma_start(wvT_sb[:, cb * 192:(cb + 1) * 192], wvT[cb * P:(cb + 1) * P, :])
        for cb in range(CB):
            nc.sync.dma_start(woT_sb[:, cb * C:(cb + 1) * C], woT[cb * P:(cb + 1) * P, :])
            nc.sync.dma_start(bo_sb[:, cb:cb + 1], bo_d[cb * P:(cb + 1) * P, :])

        # ---- Q/K projections into q_T / k_T [head-dim on partitions] ----
        qT_sb = bigp.tile([P, 2 * N], bf16, name="qT_sb")
        kT_sb = bigp.tile([P, 2 * N], bf16, name="kT_sb")
        for co, m, dst, blk in PROJ_PASSES:
            dst_sb = qT_sb if dst == "q" else kT_sb
            for qp in range(NQC // 2):
                ps = psS.tile([P, 2 * QCH], f32, name=f"pj_{dst}_{blk}_{qp}", tag="psS")
                for half in range(2):
                    qn = qp * 2 + half
                    for cb in range(CB):
                        nc.tensor.matmul(
                            ps[:m, half * QCH:(half + 1) * QCH],
                            lhsT=wqkT_sb[:, cb * 384 + co: cb * 384 + co + m],
                            rhs=xT_sb[:, cb * N + qn * QCH: cb * N + qn * QCH + QCH],
                            start=(cb == 0), stop=(cb == CB - 1),
                        )
                nc.vector.tensor_copy(
                    dst_sb[:m, blk * N + qp * 2 * QCH: blk * N + (qp + 1) * 2 * QCH],
                    ps[:m, :])

        # ---- V projection into [n, 3*(64+1)] layout with ones columns ----
        # NB: start=True clears has_written for the WHOLE psum bank, so each
        # bank may hold exactly one accumulation group: project all 3 heads
        # as one [128, 192] group, then split into the 65-strided layout.
        v_sb = bigp.tile([P, KB * VW], bf16, name="v_sb")
        for nb in range(KB):
            ps = psS.tile([P, 2 * QCH], f32, name=f"vps_{nb}", tag="psS")
            for cb in range(CB):
                nc.tensor.matmul(
                    ps[:, 0:192],
                    lhsT=xT_sb[:, cb * N + nb * P: cb * N + (nb + 1) * P],
                    rhs=wvT_sb[:, cb * 192:(cb + 1) * 192],
                    start=(cb == 0), stop=(cb == CB - 1),
                )
            # per-head [v | ones]: the ones column becomes the softmax
            # denominator row (row 64) of the PV output.  One strided copy +
            # one strided memset per key-block keeps DVE off the critical path.
            vv = v_sb[:, nb * VW:(nb + 1) * VW].rearrange("p (h w) -> p h w", h=HPC)
            pp = ps[:, 0:192].rearrange("p (h w) -> p h w", h=HPC)
            nc.vector.tensor_copy(vv[:, :, 0:64], pp[:, :, :])
            nc.vector.memset(vv[:, :, 64:65], 1.0)

        # ---- attention ----
        # Per (head, kb): 4 score matmuls (k_T[kb] stationary, reused across
        # the 4 q-chunks) -> exp -> 4 PV matmuls with v_aug[kb] stationary and
        # exp(S_T) moving.  PV output is ctx_T [d, q] (already transposed) in
        # 4 psum banks (one per q-chunk), accumulated over kb: single
        # accumulation group per bank.  The ones column of v_aug lands the
        # softmax denominator in row 64 of the same psum tile.  Normalization
        # multiplies by a PE-broadcast reciprocal row.  PV(kb-1) is emitted
        # between score matmuls of kb so the PE never starves while ACT exps.
        ctxT_sb = bigp.tile([64, 3 * N], bf16, name="ctxT_sb")  # head h at cols h*N
        ctxTf_sb = bigp.tile([P, CB * 2 * RQ], bf16, name="ctxTf_sb")

        def norm_and_a2a(h, cps):
            # normalize ctx_T[d, q] /= denom[q] (row 64), then AllToAll this
            # head (overlaps the following head's compute).
            # batched chains: all recips, then all broadcast-DMA pairs (in
            # flight concurrently), then the mults - minimizes the psum-bank
            # drain latency at the head boundary
            recs = []
            for qc in range(NQC):
                rec = smallp.tile([P, QCH], f32, name=f"rec_{h}_{qc}", tag="rec")
                nc.vector.reciprocal(rec[64:65, :], cps[qc][64:65, :])
                recs.append(rec)
            rbs = []
            for qc in range(NQC):
                # replicate the reciprocal row across partitions via a DRAM
                # round-trip (step-0 source AP); engines cannot cross lanes
                rtmp = dramp.tile([1, QCH], f32, name=f"rtmp_{h}_{qc}")
                nc.sync.dma_start(rtmp[:, :], recs[qc][64:65, :])
                rb_sb = smallp.tile([P, QCH], f32, name=f"rbsb_{h}_{qc}", tag="rbsb")
                nc.sync.dma_start(rb_sb[0:64, :], rtmp[0:1, :].partition_broadcast(64))
                rbs.append(rb_sb)
            for qc in range(NQC):
                nc.vector.tensor_mul(
                    ctxT_sb[0:64, h * N + qc * QCH: h * N + (qc + 1) * QCH],
                    cps[qc][0:64, :],
                    rbs[qc][0:64, :],
                )
            # send_h[j] = my head h ctx_T for core j's q rows; after the
            # exchange, recv_h[s] = head h of source s (batch s//4) for MY
            # q rows.  j-within-batch of (s, h, d) = (s%4)*192 + h*64 + d.
            send_h = dramp.tile([NCORES, 64, RQ], bf16, name=f"send_{h}")
            recv_h = dramp.tile([NCORES, 64, RQ], bf16, name=f"recv_{h}")
            for j in range(NCORES):
                nc.sync.dma_start(send_h[j, :, :],
                                  ctxT_sb[:, h * N + j * RQ: h * N + (j + 1) * RQ])
            nc.gpsimd.collective_compute(
                "AllToAll", mybir.AluOpType.bypass,
                replica_groups=[list(range(NCORES))],
                ins=[send_h.opt()], outs=[recv_h.opt()],
            )
            for s in range(NCORES):
                jw = (s % 4) * 192 + h * 64
                jb, ro2 = divmod(jw, P)
                co = (s // 4) * RQ
                nc.sync.dma_start(
                    ctxTf_sb[ro2:ro2 + 64, jb * 2 * RQ + co: jb * 2 * RQ + co + RQ],
                    recv_h[s, :, :])

        norm_pending = None
        for h in range(HPC):
            hb_, ho_ = HOFF[h]
            cps = [psC.tile([P, QCH], f32, name=f"cps_{h}_{qc}", tag="psC")
                   for qc in range(NQC)]
            es_hist = []

            def pv(kb, es_tiles, h=h, cps=cps):
                for qc in range(NQC):
                    nc.tensor.matmul(
                        cps[qc][0:65, :],
                        lhsT=v_sb[:, kb * VW + h * 65: kb * VW + (h + 1) * 65],
                        rhs=es_tiles[qc // 2][:, (qc % 2) * QCH:(qc % 2 + 1) * QCH],
                        start=(kb == 0), stop=(kb == KB - 1),
                    )

            for kb in range(KB):
                es_cur = []
                for qp in range(NQC // 2):
                    sps = psS.tile([P, 2 * QCH], f32, name=f"sps_{h}_{kb}_{qp}", tag="psS")
                    for half in range(2):
                        qc = qp * 2 + half
                        nc.tensor.matmul(
                            sps[:, half * QCH:(half + 1) * QCH],
                            lhsT=kT_sb[ho_:ho_ + 64, hb_ * N + kb * P: hb_ * N + (kb + 1) * P],
                            rhs=qT_sb[ho_:ho_ + 64, hb_ * N + qc * QCH: hb_ * N + qc * QCH + QCH],
                            start=True, stop=True,
                        )
                    es = esp.tile([P, 2 * QCH], bf16, name=f"es_{h}_{kb}_{qp}", tag="es")
                    nc.scalar.activation(es, sps, Exp, scale=SCALE)
                    es_cur.append(es)
                if kb == 0 and norm_pending is not None:
                    # previous head's normalization + A2A, emitted after this
                    # head's first scores so the PE pipeline never drains
                    norm_and_a2a(*norm_pending)
                    norm_pending = None
                # PV lags scores by 2 key-blocks: absorbs the previous head's
                # psum-bank drain latency without stalling the PE
                if kb >= 2:
                    pv(kb - 2, es_hist.pop(0))
                es_hist.append(es_cur)
            pv(KB - 2, es_hist.pop(0))
            pv(KB - 1, es_hist.pop(0))
            norm_pending = (h, cps)
        norm_and_a2a(*norm_pending)

        if dbg is not None:
            nc.sync.dma_start(dbg["ctxT"][:, :], ctxT_sb[:, :])
            nc.sync.dma_start(dbg["qT"][:, :], qT_sb[:, :])
            nc.sync.dma_start(dbg["kT"][:, :], kT_sb[:, :])
            nc.sync.dma_start(dbg["v"][:, :], v_sb[:, :])

        # ---- output projection (full Wo) + bias; out_T [c, 2*256] ----
        for cbo in range(CB):
            ps = psS.tile([P, 2 * QCH], f32, name=f"ops_{cbo}", tag="psS")
            for jc in range(CB):
                nc.tensor.matmul(
                    ps[:, 0:2 * RQ],
                    lhsT=woT_sb[:, jc * C + cbo * P: jc * C + (cbo + 1) * P],
                    rhs=ctxTf_sb[:, jc * 2 * RQ:(jc + 1) * 2 * RQ],
                    start=(jc == 0), stop=(jc == CB - 1),
                )
            osb = outp.tile([P, 2 * RQ], f32, name=f"osb_{cbo}", tag="osb")
            nc.scalar.activation(osb, ps[:, 0:2 * RQ], Identity, bias=bo_sb[:, cbo:cbo + 1])
            nc.sync.dma_start(out_d[cbo * P:(cbo + 1) * P, :], osb)


def build(debug_outs=False):
    nc = bacc.Bacc("TRN2", target_bir_lowering=False, debug=False, num_devices=NCORES)
    xT = nc.dram_tensor("xT", [C, N], bf16, kind="ExternalInput").ap()
    wqkT = nc.dram_tensor("wqkT", [C, 2 * HPC * HD], bf16, kind="ExternalInput").ap()
    wvT = nc.dram_tensor("wvT", [C, HPC * HD], bf16, kind="ExternalInput").ap()
    woT = nc.dram_tensor("woT", [C, C], bf16, kind="ExternalInput").ap()
    bo_d = nc.dram_tensor("bo", [C, 1], f32, kind="ExternalInput").ap()
    out_d = nc.dram_tensor("out", [C, 2 * RQ], f32, kind="ExternalOutput").ap()
    dbg = None
    if debug_outs:
        dbg = {
            "ctxT": nc.dram_tensor("dbg_ctxT", [64, 3 * N], bf16, kind="ExternalOutput").ap(),
            "qT": nc.dram_tensor("dbg_qT", [P, 2 * N], bf16, kind="ExternalOutput").ap(),
            "kT": nc.dram_tensor("dbg_kT", [P, 2 * N], bf16, kind="ExternalOutput").ap(),
            "v": nc.dram_tensor("dbg_v", [P, KB * VW], bf16, kind="ExternalOutput").ap(),
        }
    with tile.TileContext(nc) as tc:
        _body(nc, tc, xT, wqkT, wvT, woT, bo_d, out_d, dbg)
    nc.compile()
    return nc


_NC = None


def _get_nc():
    global _NC
    if _NC is None:
        _NC = build()
    return _NC


def make_in_maps(x, Wq, Wk, Wv, Wo, bo):
    x = np.asarray(x, np.float32)
    woT = np.ascontiguousarray(np.asarray(Wo, np.float32).T).astype(ml_dtypes.bfloat16)
    bo_col = np.ascontiguousarray(np.asarray(bo, np.float32).reshape(C, 1))
    in_maps = []
    for i in range(NCORES):
        b = i // 4
        hs = (i % 4) * HPC
        rq = slice(hs * HD, (hs + HPC) * HD)
        wq_s = np.asarray(Wq, np.float32)[rq]  # [192, 768]
        wk_s = np.asarray(Wk, np.float32)[rq]
        # column order matches PROJ_PASSES: [q0 q1 | k0 k1 | q2 | k2]
        wqk = np.concatenate([wq_s[0:128], wk_s[0:128], wq_s[128:192], wk_s[128:192]], axis=0).T
        in_maps.append({
            "xT": np.ascontiguousarray(x[b].T).astype(ml_dtypes.bfloat16),
            "wqkT": np.ascontiguousarray(wqk).astype(ml_dtypes.bfloat16),
            "wvT": np.ascontiguousarray(np.asarray(Wv, np.float32)[rq].T).astype(ml_dtypes.bfloat16),
            "woT": woT,
            "bo": bo_col,
        })
    return in_maps


def unshard(results):
    out = np.empty((B, N, C), np.float32)
    for i, r in enumerate(results):
        o = r["out"]  # [768, 512]: cols 0-255 batch 0, 256-511 batch 1
        out[0, i * RQ:(i + 1) * RQ, :] = o[:, :RQ].T
        out[1, i * RQ:(i + 1) * RQ, :] = o[:, RQ:].T
    return out


def kernel(x, Wq, Wk, Wv, Wo, bo):
    nc = _get_nc()
    in_maps = make_in_maps(x, Wq, Wk, Wv, Wo, bo)
    res = run_bass_kernel_spmd(nc, in_maps, core_ids=list(range(NCORES)))
    return unshard(res.results)


# revision 60
# speedup vs baseline: 1.0781x; 1.0781x over previous
"""Distributed multi-head attention kernel for 8 TRN2 NeuronCores.

Problem: B=2, N=2048, C=768, H=12 heads of dim 64.
  q = x @ Wq.T ; k = x @ Wk.T ; v = x @ Wv.T      (per-head split)
  out = softmax(q k^T / 8) v                        (full N^2 attention)
  y = concat_heads(out) @ Wo.T + bo

Sharding: 24 (batch, head) pairs -> 3 per core.  Core i owns batch i//4 and
heads 3*(i%4)..3*(i%4)+2.  Projections + attention are fully local (weights
row-sliced on the host).  An 8-way AllToAll then redistributes the per-head
context so core i owns query rows 256*i..256*(i+1) of BOTH batches with all
12 heads, after which the output projection (full Wo, replicated) produces a
disjoint output slice per core.  All matmuls run in bf16 (f32 PSUM accum).
"""

import numpy as np
import ml_dtypes

import concourse.bass as bass
import concourse.mybir as mybir
import concourse.tile as tile
from concourse import bacc
from concourse.bass_utils import run_bass_kernel_spmd

B, N, C, H, HD = 2, 2048, 768, 12, 64
SCALE = HD ** -0.5          # 0.125
P = 128
CB = C // P                 # 6 contraction blocks of 128 over channels
KB = N // P                 # 16 key blocks
QCH = 512                   # query chunk (max moving free dim)
NQC = N // QCH              # 4
HPC = 3                     # heads per core
NCORES = 8
VW = HPC * (HD + 1)         # 195: v columns per key-block (3 heads + ones col)
RQ = N // NCORES            # 256 query rows per core per batch after A2A

f32 = mybir.dt.float32
bf16 = mybir.dt.bfloat16
Exp = mybir.ActivationFunctionType.Exp
Identity = mybir.ActivationFunctionType.Identity

# head -> (block, partition offset) inside qT_sb / kT_sb [128, 2*2048].
# Identical offsets for q and k per head (PE needs matching base partitions):
# block 0 rows 0:64 = head 0, rows 64:128 = head 1; block 1 rows 0:64 = head 2.
HOFF = {0: (0, 0), 1: (0, 64), 2: (1, 0)}
# wqkT host column order: [q0 q1 | k0 k1 | q2 | k2] (projection passes)
PROJ_PASSES = [
    # (wqkT col offset, M, dest 'q' or 'k', dest block)
    (0, 128, "q", 0),
    (128, 128, "k", 0),
    (256, 64, "q", 1),
    (320, 64, "k", 1),
]


def _body(nc, tc, xT, wqkT, wvT, woT, bo_d, out_d, dbg=None):
    with (
        tc.tile_pool(name="const", bufs=1) as constp,
        tc.tile_pool(name="big", bufs=1) as bigp,
        tc.tile_pool(name="esp", bufs=12) as esp,
        tc.tile_pool(name="smallp", bufs=4) as smallp,
        tc.tile_pool(name="outp", bufs=2) as outp,
        tc.tile_pool(name="psS", bufs=2, space="PSUM") as psS,
        tc.tile_pool(name="psC", bufs=4, space="PSUM") as psC,
        tc.tile_pool(name="dram", bufs=1, space="DRAM") as dramp,
    ):
        # psS tiles are [128, 1024] f32 (2 banks); also reused for the
        # projection, broadcast, and output-projection matmuls.
        # psC: 4 single-bank PV accumulators.  Total 2*2 + 4 = 8 banks.
        # ---- load inputs to SBUF (all bf16 except bias) ----
        xT_sb = bigp.tile([P, CB * N], bf16, name="xT_sb")
        wqkT_sb = bigp.tile([P, CB * 384], bf16, name="wqkT_sb")
        wvT_sb = bigp.tile([P, CB * 192], bf16, name="wvT_sb")
        woT_sb = bigp.tile([P, CB * C], bf16, name="woT_sb")
        bo_sb = bigp.tile([P, CB], f32, name="bo_sb")
        ones_sb = constp.tile([P, 64], bf16, name="ones_sb")
        nc.vector.memset(ones_sb[:, :], 1.0)
        # warm the ACT exp table (one-time ~2.7us PSEUDO_LOAD) during loads
        warm_sb = constp.tile([P, 1], f32, name="warm_sb")
        nc.scalar.activation(warm_sb[0:1, :], ones_sb[0:1, 0:1], Exp, scale=SCALE)
        # weights for the first projection pass and x first; wo/bo last
        for cb in range(CB):
            nc.sync.dma_start(wqkT_sb[:, cb * 384:(cb + 1) * 384], wqkT[cb * P:(cb + 1) * P, :])
        for cb in range(CB):
            nc.sync.dma_start(xT_sb[:, cb * N:(cb + 1) * N], xT[cb * P:(cb + 1) * P, :])
            nc.sync.dma_start(wvT_sb[:, cb * 192:(cb + 1) * 192], wvT[cb * P:(cb + 1) * P, :])
        for cb in range(CB):
            nc.sync.dma_start(woT_sb[:, cb * C:(cb + 1) * C], woT[cb * P:(cb + 1) * P, :])
            nc.sync.dma_start(bo_sb[:, cb:cb + 1], bo_d[cb * P:(cb + 1) * P, :])

        # ---- Q/K projections into q_T / k_T [head-dim on partitions] ----
        qT_sb = bigp.tile([P, 2 * N], bf16, name="qT_sb")
        kT_sb = bigp.tile([P, 2 * N], bf16, name="kT_sb")
        for co, m, dst, blk in PROJ_PASSES:
            dst_sb = qT_sb if dst == "q" else kT_sb
            for qp in range(NQC // 2):
                ps = psS.tile([P, 2 * QCH], f32, name=f"pj_{dst}_{blk}_{qp}", tag="psS")
                for half in range(2):
                    qn = qp * 2 + half
                    for cb in range(CB):
                        nc.tensor.matmul(
                            ps[:m, half * QCH:(half + 1) * QCH],
                            lhsT=wqkT_sb[:, cb * 384 + co: cb * 384 + co + m],
                            rhs=xT_sb[:, cb * N + qn * QCH: cb * N + qn * QCH + QCH],
                            start=(cb == 0), stop=(cb == CB - 1),
                        )
                nc.vector.tensor_copy(
                    dst_sb[:m, blk * N + qp * 2 * QCH: blk * N + (qp + 1) * 2 * QCH],
                    ps[:m, :])

        # ---- V projection into [n, 3*(64+1)] layout with ones columns ----
        # NB: start=True clears has_written for the WHOLE psum bank, so each
        # bank may hold exactly one accumulation group: project all 3 heads
        # as one [128, 192] group, then split into the 65-strided layout.
        v_sb = bigp.tile([P, KB * VW], bf16, name="v_sb")
        for nb in range(KB):
            ps = psS.tile([P, 2 * QCH], f32, name=f"vps_{nb}", tag="psS")
            for cb in range(CB):
                nc.tensor.matmul(
                    ps[:, 0:192],
                    lhsT=xT_sb[:, cb * N + nb * P: cb * N + (nb + 1) * P],
                    rhs=wvT_sb[:, cb * 192:(cb + 1) * 192],
                    start=(cb == 0), stop=(cb == CB - 1),
                )
            # per-head [v | ones]: the ones column becomes the softmax
            # denominator row (row 64) of the PV output.  One strided copy +
            # one strided memset per key-block keeps DVE off the critical path.
            vv = v_sb[:, nb * VW:(nb + 1) * VW].rearrange("p (h w) -> p h w", h=HPC)
            pp = ps[:, 0:192].rearrange("p (h w) -> p h w", h=HPC)
            nc.vector.tensor_copy(vv[:, :, 0:64], pp[:, :, :])
            nc.vector.memset(vv[:, :, 64:65], 1.0)

        # ---- attention ----
        # Per (head, kb): 4 score matmuls (k_T[kb] stationary, reused across
        # the 4 q-chunks) -> exp -> 4 PV matmuls with v_aug[kb] stationary and
        # exp(S_T) moving.  PV output is ctx_T [d, q] (already transposed) in
        # 4 psum banks (one per q-chunk), accumulated over kb: single
        # accumulation group per bank.  The ones column of v_aug lands the
        # softmax denominator in row 64 of the same psum tile.  Normalization
        # multiplies by a DMA-broadcast reciprocal row.  PV lags the scores by
        # 2 key-blocks so the PE never starves while ACT runs exp, and the
        # previous head's psum banks have time to drain at head boundaries.
        PVLAG = 4
        ctxT_sb = bigp.tile([64, 3 * N], bf16, name="ctxT_sb")  # head h at cols h*N
        ctxTf_sb = bigp.tile([P, CB * 2 * RQ], bf16, name="ctxTf_sb")

        def norm_and_a2a(h, cps):
            # normalize ctx_T[d, q] /= denom[q] (row 64), then AllToAll this
            # head (overlaps the following head's compute).
            for qc in range(NQC):
                rec = smallp.tile([P, QCH], f32, name=f"rec_{h}_{qc}", tag="rec")
                nc.vector.reciprocal(rec[64:65, :], cps[qc][64:65, :])
                # replicate the reciprocal row across partitions via a DRAM
                # round-trip (step-0 source AP); engines cannot cross lanes
                rtmp = dramp.tile([1, QCH], f32, name=f"rtmp_{h}_{qc}")
                nc.sync.dma_start(rtmp[:, :], rec[64:65, :])
                rb_sb = smallp.tile([P, QCH], f32, name=f"rbsb_{h}_{qc}", tag="rbsb")
                nc.sync.dma_start(rb_sb[0:64, :], rtmp[0:1, :].partition_broadcast(64))
                nc.vector.tensor_mul(
                    ctxT_sb[0:64, h * N + qc * QCH: h * N + (qc + 1) * QCH],
                    cps[qc][0:64, :],
                    rb_sb[0:64, :],
                )
            # send_h[j] = my head h ctx_T for core j's q rows; after the
            # exchange, recv_h[s] = head h of source s (batch s//4) for MY
            # q rows.  j-within-batch of (s, h, d) = (s%4)*192 + h*64 + d.
            send_h = dramp.tile([NCORES, 64, RQ], bf16, name=f"send_{h}")
            recv_h = dramp.tile([NCORES, 64, RQ], bf16, name=f"recv_{h}")
            for j in range(NCORES):
                nc.sync.dma_start(send_h[j, :, :],
                                  ctxT_sb[:, h * N + j * RQ: h * N + (j + 1) * RQ])
            nc.gpsimd.collective_compute(
                "AllToAll", mybir.AluOpType.bypass,
                replica_groups=[list(range(NCORES))],
                ins=[send_h.opt()], outs=[recv_h.opt()],
            )
            for s in range(NCORES):
                jw = (s % 4) * 192 + h * 64
                jb, ro2 = divmod(jw, P)
                co = (s // 4) * RQ
                nc.sync.dma_start(
                    ctxTf_sb[ro2:ro2 + 64, jb * 2 * RQ + co: jb * 2 * RQ + co + RQ],
                    recv_h[s, :, :])

        norm_pending = None
        for h in range(HPC):
            hb_, ho_ = HOFF[h]
            cps = [psC.tile([P, QCH], f32, name=f"cps_{h}_{qc}", tag="psC")
                   for qc in range(NQC)]
            es_hist = []

            def pv(kb, es_tiles, h=h, cps=cps):
                for qc in range(NQC):
                    nc.tensor.matmul(
                        cps[qc][0:65, :],
                        lhsT=v_sb[:, kb * VW + h * 65: kb * VW + (h + 1) * 65],
                        rhs=es_tiles[qc // 2][:, (qc % 2) * QCH:(qc % 2 + 1) * QCH],
                        start=(kb == 0), stop=(kb == KB - 1),
                    )

            for kb in range(KB):
                es_cur = []
                for qp in range(NQC // 2):
                    sps = psS.tile([P, 2 * QCH], f32, name=f"sps_{h}_{kb}_{qp}", tag="psS")
                    for half in range(2):
                        qc = qp * 2 + half
                        nc.tensor.matmul(
                            sps[:, half * QCH:(half + 1) * QCH],
                            lhsT=kT_sb[ho_:ho_ + 64, hb_ * N + kb * P: hb_ * N + (kb + 1) * P],
                            rhs=qT_sb[ho_:ho_ + 64, hb_ * N + qc * QCH: hb_ * N + qc * QCH + QCH],
                            start=True, stop=True,
                        )
                    es = esp.tile([P, 2 * QCH], bf16, name=f"es_{h}_{kb}_{qp}", tag="es")
                    nc.scalar.activation(es, sps, Exp, scale=SCALE)
                    es_cur.append(es)
                if kb == 0 and norm_pending is not None:
                    # previous head's normalization + A2A, emitted after this
                    # head's first scores so the PE pipeline never drains
                    norm_and_a2a(*norm_pending)
                    norm_pending = None
                # PV lags scores by PVLAG key-blocks: absorbs the previous
                # head's psum-bank drain latency without stalling the PE
                if kb >= PVLAG:
                    pv(kb - PVLAG, es_hist.pop(0))
                es_hist.append(es_cur)
            for kk in range(KB - PVLAG, KB):
                pv(kk, es_hist.pop(0))
            norm_pending = (h, cps)
        norm_and_a2a(*norm_pending)

        if dbg is not None:
            nc.sync.dma_start(dbg["ctxT"][:, :], ctxT_sb[:, :])
            nc.sync.dma_start(dbg["qT"][:, :], qT_sb[:, :])
            nc.sync.dma_start(dbg["kT"][:, :], kT_sb[:, :])
            nc.sync.dma_start(dbg["v"][:, :], v_sb[:, :])

        # ---- output projection (full Wo) + bias; out_T [c, 2*256] ----
        for cbo in range(CB):
            ps = psS.tile([P, 2 * QCH], f32, name=f"ops_{cbo}", tag="psS")
            for jc in range(CB):
                nc.tensor.matmul(
                    ps[:, 0:2 * RQ],
                    lhsT=woT_sb[:, jc * C + cbo * P: jc * C + (cbo + 1) * P],
                    rhs=ctxTf_sb[:, jc * 2 * RQ:(jc + 1) * 2 * RQ],
                    start=(jc == 0), stop=(jc == CB - 1),
                )
            osb = outp.tile([P, 2 * RQ], f32, name=f"osb_{cbo}", tag="osb")
            nc.scalar.activation(osb, ps[:, 0:2 * RQ], Identity, bias=bo_sb[:, cbo:cbo + 1])
            nc.sync.dma_start(out_d[cbo * P:(cbo + 1) * P, :], osb)


def build(debug_outs=False):
    nc = bacc.Bacc("TRN2", target_bir_lowering=False, debug=False, num_devices=NCORES)
    xT = nc.dram_tensor("xT", [C, N], bf16, kind="ExternalInput").ap()
    wqkT = nc.dram_tensor("wqkT", [C, 2 * HPC * HD], bf16, kind="ExternalInput").ap()
    wvT = nc.dram_tensor("wvT", [C, HPC * HD], bf16, kind="ExternalInput").ap()
    woT = nc.dram_tensor("woT", [C, C], bf16, kind="ExternalInput").ap()
    bo_d = nc.dram_tensor("bo", [C, 1], f32, kind="ExternalInput").ap()
    out_d = nc.dram_tensor("out", [C, 2 * RQ], f32, kind="ExternalOutput").ap()
    dbg = None
    if debug_outs:
        dbg = {
            "ctxT": nc.dram_tensor("dbg_ctxT", [64, 3 * N], bf16, kind="ExternalOutput").ap(),
            "qT": nc.dram_tensor("dbg_qT", [P, 2 * N], bf16, kind="ExternalOutput").ap(),
            "kT": nc.dram_tensor("dbg_kT", [P, 2 * N], bf16, kind="ExternalOutput").ap(),
            "v": nc.dram_tensor("dbg_v", [P, KB * VW], bf16, kind="ExternalOutput").ap(),
        }
    with tile.TileContext(nc) as tc:
        _body(nc, tc, xT, wqkT, wvT, woT, bo_d, out_d, dbg)
    nc.compile()
    return nc


_NC = None


def _get_nc():
    global _NC
    if _NC is None:
        _NC = build()
    return _NC


def make_in_maps(x, Wq, Wk, Wv, Wo, bo):
    x = np.asarray(x, np.float32)
    woT = np.ascontiguousarray(np.asarray(Wo, np.float32).T).astype(ml_dtypes.bfloat16)
    bo_col = np.ascontiguousarray(np.asarray(bo, np.float32).reshape(C, 1))
    in_maps = []
    for i in range(NCORES):
        b = i // 4
        hs = (i % 4) * HPC
        rq = slice(hs * HD, (hs + HPC) * HD)
        wq_s = np.asarray(Wq, np.float32)[rq]  # [192, 768]
        wk_s = np.asarray(Wk, np.float32)[rq]
        # column order matches PROJ_PASSES: [q0 q1 | k0 k1 | q2 | k2]
        wqk = np.concatenate([wq_s[0:128], wk_s[0:128], wq_s[128:192], wk_s[128:192]], axis=0).T
        in_maps.append({
            "xT": np.ascontiguousarray(x[b].T).astype(ml_dtypes.bfloat16),
            "wqkT": np.ascontiguousarray(wqk).astype(ml_dtypes.bfloat16),
            "wvT": np.ascontiguousarray(np.asarray(Wv, np.float32)[rq].T).astype(ml_dtypes.bfloat16),
            "woT": woT,
            "bo": bo_col,
        })
    return in_maps


def unshard(results):
    out = np.empty((B, N, C), np.float32)
    for i, r in enumerate(results):
        o = r["out"]  # [768, 512]: cols 0-255 batch 0, 256-511 batch 1
        out[0, i * RQ:(i + 1) * RQ, :] = o[:, :RQ].T
        out[1, i * RQ:(i + 1) * RQ, :] = o[:, RQ:].T
    return out


def kernel(x, Wq, Wk, Wv, Wo, bo):
    nc = _get_nc()
    in_maps = make_in_maps(x, Wq, Wk, Wv, Wo, bo)
    res = run_bass_kernel_spmd(nc, in_maps, core_ids=list(range(NCORES)))
    return unshard(res.results)
